# revision 26
# baseline (speedup 1.0000x reference)
"""GQA attention (dense transformer block) on 8 TRN2 NeuronCores — v2.

Tensor-parallel over heads for QKV+attention: core c owns Q heads 4c..4c+3
and KV head c; projections + RoPE + causal attention are fully local.

Output projection is row-sharded over sequence: each core computes the FULL
4096-dim output for 256 of the 2048 seq rows (4 blocks of 64: one per sq
quarter).  Attention outputs are redistributed with four small AllToAlls
(one per 512-wide sq quarter, 512KB each) instead of the v1 AllGathers
(5 x 25-40us on HW): per-core collective traffic drops 8x and all but the
last A2A overlap compute.  wo is streamed from HBM (full 32MB fp16) during
the tail at 512KB/k-tile against 8 PSUM accumulator banks, so no SBUF
output accumulator and no DVE adds on the wo path.

All matmuls fp16 (full PE rate), fp32 PSUM.  Softmax skips max-subtraction
(scores ~N(0,1), max ~9-12; constant -5 bias inside exp cancels in the
ratio).  Scores are computed transposed [sk, sq] in 512-wide sq quarters so
exp'd tiles feed PV directly as rhs; softmax denominators via a ones-column
matmul; 1/r via the fast custom-DVE reciprocal (the stock DVE RECIPROCAL
measured 6.5us per call on HW).

Scheduling: attention quarter q is emitted inside phase-1 window-pair q+1,
so its ACT-bound exp hides under projection matmuls; A2A(q) fires as soon
as all 4 heads' quarter-q outputs are stored.  Phase-1 DMA order puts the
first two x windows and cos/sin ahead of wq (split in 4 head-chunks) so the
k/v chains start ~6us in and q chains are never weight-starved.
"""

import sys

import numpy as np

sys.path.insert(0, "/opt/trn_rl_repo")

S = 2048          # sequence length
D = 4096          # model dim
HD = 128          # head dim
NCORE = 8
QW = 256          # seq window for QKV projection (x streamed in these)
NQW = S // QW     # 8
KD = D // 128     # 32 contraction tiles over model dim
AW = 512          # attention sq quarter-window
NAW = S // AW     # 4
NH_LOC = 4        # local q heads per core
RB = 128          # seq rows per (core, half) for the output projection
SCALE = 1.0 / float(np.sqrt(128.0))

# even dims then odd dims, within one head: makes RoPE's interleaved
# pairing contiguous (x1 = partitions 0:64, x2 = partitions 64:128)
_PERM_EO = np.concatenate([np.arange(0, 128, 2), np.arange(1, 128, 2)])

_GRAPH = None
_MARKS = []
USE_FAST_RECIP = False
MOCK_CC = False


def _build_graph(dbg=False, repeat=1, sim_mode=False):
    """sim_mode: replace collectives with local DRAM->DRAM copies so the
    single-core TimelineSim can schedule the graph (timing study only --
    results are wrong for cores != 0)."""
    import concourse.bacc as bacc
    import concourse.mybir as mybir
    import concourse.tile as tile

    F16 = mybir.dt.float16
    F32 = mybir.dt.float32
    RG = [list(range(NCORE))]
    Exp = mybir.ActivationFunctionType.Exp

    nc = bacc.Bacc(
        "TRN2", target_bir_lowering=False, debug=False, num_devices=NCORE
    )

    # x windows pre-arranged in SBUF layout [w][p 128][k 32][n 256]
    xtw = nc.dram_tensor("xtw", [NQW, 128, KD * QW], F16, kind="ExternalInput").ap()
    # wq head-major, SBUF-layout: [p 128, (h 4, k 32, n 128)]
    wqt = nc.dram_tensor("wqt", [128, NH_LOC * KD * 128], F16, kind="ExternalInput").ap()
    # wkv pre-arranged in SBUF layout [p 128][k 32][n 256]
    wkvt = nc.dram_tensor("wkvt", [128, KD * 256], F16, kind="ExternalInput").ap()
    # full output projection, transposed: [kt, 128 (contraction), 4096 (out)]
    wot = nc.dram_tensor("wot", [KD, 128, D], F16, kind="ExternalInput").ap()
    cost = nc.dram_tensor("cost", [128, S], F16, kind="ExternalInput").ap()
    sgnt = nc.dram_tensor("sgnt", [128, S], F16, kind="ExternalInput").ap()
    band = nc.dram_tensor("band", [128, AW], F16, kind="ExternalInput").ap()
    onesv = nc.dram_tensor("onesv", [128, 128], F16, kind="ExternalInput").ap()
    ident = nc.dram_tensor("ident", [128, 128], F16, kind="ExternalInput").ap()
    # out rows (per core c): rt*1024 + q'*512 + 64c .. +64  for rt in {0,1},
    # q' in {0,1}; tile rt covers quarters 2rt (cols 0:64) and 2rt+1 (64:128)
    out_ext = nc.dram_tensor("out", [2, 128, D], F16, kind="ExternalOutput").ap()

    from contextlib import ExitStack

    def mark(label):
        _MARKS.append((label, nc.next_id()))

    with tile.TileContext(nc) as tc, ExitStack() as ctx:
        ec = ctx.enter_context
        wpool = ec(tc.tile_pool(name="wpool", bufs=1))
        xpool = ec(tc.tile_pool(name="xpool", bufs=3))
        qkvp = ec(tc.tile_pool(name="qkvp", bufs=1))
        rp = ec(tc.tile_pool(name="rp", bufs=2))
        vtp = ec(tc.tile_pool(name="vtp", bufs=2))
        ptp = ec(tc.tile_pool(name="ptp", bufs=3))
        racp = ec(tc.tile_pool(name="racp", bufs=2))
        rcbp = ec(tc.tile_pool(name="rcbp", bufs=2))
        aop = ec(tc.tile_pool(name="aop", bufs=2))
        gp = ec(tc.tile_pool(name="gp", bufs=1))
        wsp = ec(tc.tile_pool(name="wsp", bufs=8))
        osb = ec(tc.tile_pool(name="osb", bufs=2))
        # PSUM: 8 banks of [128, 512] f32 equivalents:
        #   mmp (2) + stp (2) + pvp (1) + wop (3)
        mmp = ec(tc.tile_pool(name="mmp", bufs=3, space="PSUM"))
        stp = ec(tc.tile_pool(name="stp", bufs=2, space="PSUM"))
        # pvp bufs=2: consecutive attention blocks would otherwise
        # serialize on the single pv slot (WAR until the ao mul reads it)
        pvp = ec(tc.tile_pool(name="pvp", bufs=2, space="PSUM"))
        wop = ec(tc.tile_pool(name="wop", bufs=1, space="PSUM"))
        dramp = ec(tc.tile_pool(name="dramp", bufs=1, space="DRAM"))
        for _rep in range(repeat):
            # ---------- persistent SBUF: weights / constants ----------
            # one FIFO DMA ring: loads emitted in first-use order.
            # wkv and xw0 are interleaved in k-tile halves so the first k
            # chain starts after ~half the 4MB instead of all of it.
            wkv_sb = wpool.tile([128, KD * 256], F16, tag="wkv_sb", name="wkv_sb")
            xw_tiles = {}

            def load_xw(w, halves=False):
                xw = xpool.tile([128, KD * QW], F16, tag="xw", name="xw")
                xw_tiles[w] = xw
                if not halves:
                    nc.sync.dma_start(xw[:], xtw[w])
                    return

            def load_half(a):
                ks = slice(a * (KD // 2) * 256, (a + 1) * (KD // 2) * 256)
                nc.sync.dma_start(wkv_sb[:, ks], wkvt[:, ks])
                nc.sync.dma_start(xw_tiles[0][:, ks], xtw[0][:, ks])

            load_xw(0, halves=True)
            load_half(0)
            load_half(1)
            load_xw(1)

            cos_sb = wpool.tile([128, S], F16, tag="cos_sb", name="cos_sb")
            nc.sync.dma_start(cos_sb[:], cost[:])
            sgn_sb = wpool.tile([128, S], F16, tag="sgn_sb", name="sgn_sb")
            nc.sync.dma_start(sgn_sb[:], sgnt[:])
            band_sb = wpool.tile([128, AW], F16, tag="band_sb", name="band_sb")
            nc.sync.dma_start(band_sb[:], band[:])
            ones_sb = wpool.tile([128, 128], F16, tag="ones_sb", name="ones_sb")
            nc.sync.dma_start(ones_sb[:], onesv[:])
            id_sb = wpool.tile([128, 128], F16, tag="id_sb", name="id_sb")
            nc.sync.dma_start(id_sb[:], ident[:])
            b5_sb = wpool.tile([128, 1], F32, tag="b5_sb", name="b5_sb")
            nc.vector.memset(b5_sb[:], -5.0)
            # wq split into 4 head-chunks so the first q chain isn't
            # starved behind the whole 4MB load; host supplies head-major
            # [p, h, k, n] so each chunk is one contiguous full-rate DMA
            wq_sb = wpool.tile([128, NH_LOC * KD * 128], F16, tag="wq_sb", name="wq_sb")
            for h in range(NH_LOC):
                hs = slice(h * KD * 128, (h + 1) * KD * 128)
                nc.sync.dma_start(wq_sb[:, hs], wqt[:, hs])

            # persistent QKV results
            q_sb = [qkvp.tile([128, S], F16, tag=f"q{h}", name=f"q{h}") for h in range(NH_LOC)]
            k_sb = qkvp.tile([128, S], F16, tag="k_sb", name="k_sb")   # kT: [hd, sk]
            v_sb = qkvp.tile([128, S], F16, tag="v_sb", name="v_sb")   # v natural: [sk%128, (stile, hd)]

            # A2A bounce buffers: per sq HALF, [dest 8][h 4][p 128][s 128]
            # (1MB A2As run ~17us on HW; 512KB ones are latency-bound ~28us)
            cc_in = [
                dramp.tile([NCORE, NH_LOC * 128 * RB], F16, tag=f"cci{v}",
                           name=f"cci{v}")
                for v in range(2)
            ]
            cc_out = [
                dramp.tile([NCORE, NH_LOC * 128 * RB], F16, tag=f"cco{v}",
                           name=f"cco{v}")
                for v in range(2)
            ]

            # gt[rt]: aoT for this core's 128 rows of row-tile rt
            # (quarters 2rt / 2rt+1), laid [128 p, (kt 32, s 128)]
            gt = [
                gp.tile([128, KD * 128], F16, tag=f"g{rt}", name=f"g{rt}")
                for rt in range(2)
            ]

            def a2a(v):
                if sim_mode or MOCK_CC:
                    for j in range(NCORE):
                        nc.sync.dma_start(cc_out[v][j], cc_in[v][j])
                else:
                    nc.gpsimd.collective_compute(
                        "AllToAll",
                        mybir.AluOpType.bypass,
                        replica_groups=RG,
                        ins=[cc_in[v][:].opt()],
                        outs=[cc_out[v][:].opt()],
                    )
                # pull this half's aoT into the wo lhsT tile.  SWDGE ring:
                # on the sync FIFO this load (gated on the collective)
                # would head-of-line block every wo_t stream load behind it
                nc.gpsimd.dma_start(
                    out=gt[v][:].rearrange("p (k s) -> p k s", s=128),
                    in_=cc_out[v][:].rearrange(
                        "j (h p s) -> p (j h) s", h=NH_LOC, p=128
                    ),
                )

            def rope(ps, dst, w):
                """Apply interleaved RoPE to a [128, QW] psum tile (f32) and
                write fp16 into dst[:, w*QW:(w+1)*QW].  High priority: rope
                frees the chain's PSUM mm slot, and must not queue on DVE
                behind exp-gated attention ops."""
                cw = slice(w * QW, (w + 1) * QW)
                t1 = rp.tile([128, QW], F32, tag="t1", name="t1")
                t2 = rp.tile([128, QW], F32, tag="t2", name="t2")
                with tc.high_priority():
                    nc.vector.tensor_mul(t1[:], ps[:], cos_sb[:, cw])
                    nc.vector.tensor_mul(t2[0:64, :], ps[64:128, :], sgn_sb[0:64, cw])
                    nc.vector.tensor_mul(t2[64:128, :], ps[0:64, :], sgn_sb[64:128, cw])
                    nc.vector.tensor_add(dst[:, cw], t1[:], t2[:])

            # ---------- attention (one sq quarter at a time) ----------
            def attn_q(h, q):
                """Attention for head h, sq quarter q (512 wide).  Writes the
                fp16 attnT [128, 512] into cc_in[q] block column for head h,
                split into 8 x 64-col dest blocks."""
                base = q * AW
                nsk = (AW // 128) * (q + 1)  # causal: sk tiles 0..nsk-1
                pv = pvp.tile([128, AW], F32, tag="pv", name="pv")
                racc = racp.tile([128, AW], F16, tag="racc", name="racc")

                def st_exp(i):
                    lo = max(128 * i - base, 0)
                    st = stp.tile([128, AW], F32, tag="st", name="st")
                    nc.tensor.matmul(
                        st[:, lo:],
                        lhsT=k_sb[:, i * 128:(i + 1) * 128],
                        rhs=q_sb[h][:, base + lo:base + AW],
                        start=True,
                        stop=True,
                    )
                    pt = ptp.tile([128, AW], F16, tag="pt", name="pt")
                    # bias -5 rescales every exp by e^-5 (cancels in the
                    # softmax ratio) so fp16 holds scores up to z ~ 16
                    nc.scalar.activation(pt[:, lo:], st[:, lo:], Exp, scale=SCALE, bias=b5_sb[:])
                    if 128 * i >= base:  # diagonal tile: causal band mask
                        nc.vector.tensor_mul(
                            pt[:, lo:], pt[:, lo:], band_sb[:, 0:AW - lo],
                        )
                    if i == 0:
                        nc.vector.tensor_copy(racc[:], pt[:])
                    else:
                        nc.vector.tensor_add(racc[:, lo:], racc[:, lo:], pt[:, lo:])
                    return (pt, lo)

                # Software-pipeline by 2: emit ST_{i+2} before PV_i so the
                # exp of step i hides under the score matmuls of steps i+1/2
                LA = 2
                pts = [None] * nsk
                for i in range(min(LA, nsk)):
                    pts[i] = st_exp(i)
                for i in range(nsk):
                    if i + LA < nsk:
                        pts[i + LA] = st_exp(i + LA)
                    pt_i, lo_i = pts[i]
                    pts[i] = None
                    nc.tensor.matmul(
                        pv[:, lo_i:],
                        lhsT=v_sb[:, i * 128:(i + 1) * 128],
                        rhs=pt_i[:, lo_i:],
                        start=(i == 0),
                        stop=(i == nsk - 1),
                    )
                # softmax denominator, summed over partitions AND
                # replicated to all 128 rows in one matmul
                rb = stp.tile([128, AW], F32, tag="st", name="rb")
                nc.tensor.matmul(
                    rb[:], lhsT=ones_sb[:], rhs=racc[:], start=True, stop=True
                )
                rcb = rcbp.tile([128, AW], F32, tag="rcb", name="rcb")
                if USE_FAST_RECIP:
                    nc.vector.reciprocal_approx_fast(out=rcb[:], in_=rb[:])
                else:
                    with nc.allow_low_precision(reason="softmax 1/r; r~O(10)"):
                        nc.vector.reciprocal(rcb[:], rb[:])
                ao = aop.tile([128, AW], F16, tag="ao", name="ao")
                nc.vector.tensor_mul(ao[:], pv[:], rcb[:])
                # scatter into the A2A input: quarter q covers dest
                # blocks 4u..4u+3 of half v (128-row blocks, 256B runs)
                v, u = divmod(q, 2)
                nc.sync.dma_start(
                    cc_in[v][:].rearrange(
                        "j (h p s) -> h p j s", h=NH_LOC, p=128
                    )[h][:, 4 * u:4 * u + 4, :],
                    ao[:].rearrange("p (j s) -> p j s", s=RB),
                )

            # ---------- phase 1: QKV projections + RoPE ----------
            # window pairs; attention quarter wp-1 rides inside pair wp,
            # one head-block between consecutive chains so a stalled ST
            # never head-of-line blocks the PE queue
            for wp in range(NQW // 2):
                pair = (2 * wp, 2 * wp + 1)
                qq = wp - 1
                def attn_slot(slot, _qq=qq):
                    # quarter q2 only needs sk windows 0-5: run it in wp3
                    if 0 <= _qq <= 2 and slot < NH_LOC:
                        mark(f"attn-h{slot}-q{_qq}")
                        attn_q(slot, _qq)
                        if slot == NH_LOC - 1 and _qq == 1:
                            a2a(0)
                for w in (2 * wp + 2, 2 * wp + 3):
                    if 2 <= w < NQW:
                        load_xw(w)
                for w in pair:
                    xw = xw_tiles[w]
                    mark(f"kv-chain-w{w}")
                    # kT (RoPE'd): [hd, s]
                    ps = mmp.tile([128, QW], F32, tag="mm", name="mm")
                    for k in range(KD):
                        nc.tensor.matmul(
                            ps[:],
                            lhsT=wkv_sb[:, k * 256:k * 256 + 128],
                            rhs=xw[:, k * QW:(k + 1) * QW],
                            start=(k == 0),
                            stop=(k == KD - 1),
                        )
                    rope(ps, k_sb, w)

                    # vT: [hd, s] then PE-transpose into v natural [s, hd]
                    ps = mmp.tile([128, QW], F32, tag="mm", name="mm")
                    for k in range(KD):
                        nc.tensor.matmul(
                            ps[:],
                            lhsT=wkv_sb[:, k * 256 + 128:(k + 1) * 256],
                            rhs=xw[:, k * QW:(k + 1) * QW],
                            start=(k == 0),
                            stop=(k == KD - 1),
                        )
                    vt = vtp.tile([128, QW], F16, tag="vt", name="vt")
                    nc.vector.tensor_copy(vt[:], ps[:])
                    for t in range(QW // 128):
                        st_idx = w * (QW // 128) + t
                        tp = stp.tile([128, 128], F16, tag="st", name="tp")
                        nc.tensor.transpose(
                            tp[:], vt[:, t * 128:(t + 1) * 128], id_sb[:]
                        )
                        nc.vector.tensor_copy(
                            v_sb[:, st_idx * 128:(st_idx + 1) * 128], tp[:]
                        )
                    attn_slot(w - pair[0])
                for w in pair:
                    xw = xw_tiles[w]
                    mark(f"q-chain-w{w}")
                    # qT (RoPE'd): 4 local heads (wq_sb is head-major)
                    for h in range(NH_LOC):
                        ps = mmp.tile([128, QW], F32, tag="mm", name="mm")
                        for k in range(KD):
                            nc.tensor.matmul(
                                ps[:],
                                lhsT=wq_sb[:, (h * KD + k) * 128:(h * KD + k + 1) * 128],
                                rhs=xw[:, k * QW:(k + 1) * QW],
                                start=(k == 0),
                                stop=(k == KD - 1),
                            )
                        rope(ps, q_sb[h], w)
                    attn_slot(2 + (w - pair[0]))

                mark(f"wp{wp}-chains-done")

            # last quarter + its A2A (the only exposed collective)
            for h in range(NH_LOC):
                mark(f"attn-h{h}-q3")
                attn_q(h, NAW - 1)
            a2a(1)
            mark("wo-start")

            # ---------- output projection: rows 64c + q*512 ----------
            # stream full wo; 8 PSUM accumulator banks = {rt 2} x {n 4},
            # borrowing every pool's slots (attention is done by now)
            for H in range(2):  # out-column halves of 2048
                slots = [
                    mmp.tile([128, 512], F32, tag="mm", name="wo_ps0"),
                    mmp.tile([128, 512], F32, tag="mm", name="wo_ps1"),
                    mmp.tile([128, 512], F32, tag="mm", name="wo_ps2"),
                    stp.tile([128, 512], F32, tag="st", name="wo_ps3"),
                    stp.tile([128, 512], F32, tag="st", name="wo_ps4"),
                    pvp.tile([128, 512], F32, tag="pv", name="wo_ps5"),
                    pvp.tile([128, 512], F32, tag="pv", name="wo_ps6"),
                    wop.tile([128, 512], F32, tag="w0", name="wo_ps7"),
                ]
                ps8 = [slots[0:4], slots[4:8]]
                # rt-outer: rt0's pass starts while the last A2A (whose
                # data only rt1 needs) is still in flight; the shared wo_t
                # tiles (wsp bufs) zipper the two passes ~8 k-tiles apart
                wo_tiles = {}
                for rt in range(2):
                    for kt in range(KD):
                        if kt % 8 == 0:
                            mark(f"wo-H{H}-rt{rt}-kt{kt}")
                        if rt == 0:
                            wo_t = wsp.tile([128, 2048], F16, tag="wo_t", name="wo_t")
                            nc.sync.dma_start(wo_t[:], wot[kt, :, H * 2048:(H + 1) * 2048])
                            wo_tiles[kt] = wo_t
                        else:
                            wo_t = wo_tiles.pop(kt)
                        for n in range(4):
                            nc.tensor.matmul(
                                ps8[rt][n][:],
                                lhsT=gt[rt][:, kt * 128:(kt + 1) * 128],
                                rhs=wo_t[:, n * 512:(n + 1) * 512],
                                start=(kt == 0),
                                stop=(kt == KD - 1),
                            )
                for rt in range(2):
                    ot = osb.tile([128, 2048], F16, tag="ot", name="ot")
                    for n in range(4):
                        nc.vector.tensor_copy(
                            ot[:, n * 512:(n + 1) * 512], ps8[rt][n][:]
                        )
                    nc.sync.dma_start(
                        out_ext[rt, :, H * 2048:(H + 1) * 2048], ot[:]
                    )

    nc.compile()
    return nc


def _prep_shared(x, cos, sin, wo):
    xT = np.ascontiguousarray(x.reshape(S, D).T)  # [D, S]
    # SBUF layout per window: [p 128, (k 32, n 256)]
    xtw = np.ascontiguousarray(
        xT.reshape(KD, 128, NQW, QW).transpose(2, 1, 0, 3).reshape(
            NQW, 128, KD * QW)
    ).astype(np.float16)
    cosT = cos.T.astype(np.float32)  # [64, S]
    sinT = sin.T.astype(np.float32)
    cost = np.concatenate([cosT, cosT], 0).astype(np.float16)
    sgnt = np.concatenate([-sinT, sinT], 0).astype(np.float16)
    band = (
        np.arange(AW)[None, :] >= np.arange(128)[:, None]
    ).astype(np.float16)
    onesv = np.ones((128, 128), np.float16)
    ident = np.eye(128, dtype=np.float16)
    # full wo, transposed to [contraction (head dims), out], k-tiled
    wot = np.ascontiguousarray(wo.T).astype(np.float16).reshape(KD, 128, D)
    return xtw, cost, sgnt, band, onesv, ident, wot


def _prep_core(c, wq, wk, wv):
    # wqt: head-major SBUF layout [p 128, (h, k, n 128)]
    qrows = np.concatenate([512 * c + 128 * h + _PERM_EO for h in range(NH_LOC)])
    A = wq[qrows, :].reshape(NH_LOC, 128, KD, 128)  # h, n, (k, p)->k, p
    wqt = np.ascontiguousarray(
        A.transpose(3, 0, 2, 1).reshape(128, NH_LOC * KD * 128)
    ).astype(np.float16)
    krows = 128 * c + _PERM_EO
    wkt = wk[krows, :].T  # [D, 128]
    wvt = wv[128 * c:128 * (c + 1), :].T
    # SBUF layout: [p 128, (k 32, n 256)] with n = [k-head dims | v-head dims]
    kv = np.concatenate([wkt, wvt], axis=1).reshape(KD, 128, 256)
    wkvt = np.ascontiguousarray(kv.transpose(1, 0, 2).reshape(128, KD * 256)
                                ).astype(np.float16)
    return wqt, wkvt


def _make_in_maps(inputs):
    x = np.asarray(inputs["x"], np.float32)
    cos = np.asarray(inputs["cos"], np.float32)
    sin = np.asarray(inputs["sin"], np.float32)
    wq = np.asarray(inputs["wq"], np.float32)
    wk = np.asarray(inputs["wk"], np.float32)
    wv = np.asarray(inputs["wv"], np.float32)
    wo = np.asarray(inputs["wo"], np.float32)

    xtw, cost, sgnt, band, onesv, ident, wot = _prep_shared(x, cos, sin, wo)
    in_maps = []
    for c in range(NCORE):
        wqt, wkvt = _prep_core(c, wq, wk, wv)
        in_maps.append(
            dict(
                xtw=xtw, cost=cost, sgnt=sgnt, band=band, onesv=onesv,
                ident=ident, wot=wot, wqt=wqt, wkvt=wkvt,
            )
        )
    return in_maps


def _assemble(outs):
    """outs[c]: [2, 128, 4096]; row-tile v = seq rows [1024v+128c, +128)"""
    full = np.zeros((S, D), np.float32)
    for c in range(NCORE):
        o = np.asarray(outs[c], np.float32)
        for v in range(2):
            r0 = 1024 * v + RB * c
            full[r0:r0 + RB, :] = o[v]
    return full.reshape(1, S, D)


def _run(inputs, trace=False, dbg=False):
    global _GRAPH
    in_maps = _make_in_maps(inputs)

    if _GRAPH is None:
        _GRAPH = _build_graph()
    graph = _GRAPH

    from concourse.bass_utils import run_bass_kernel_spmd

    res = run_bass_kernel_spmd(
        graph, in_maps, core_ids=list(range(NCORE)), trace=trace
    )
    full = _assemble([res.results[c]["out"] for c in range(NCORE)])
    return full, res


def kernel(**inputs):
    full, _ = _run(inputs, trace=False)
    return full
